# revision 19
# baseline (speedup 1.0000x reference)
"""Binarized ResNet BasicBlock (conv1 3x3/s2 + BN + sign, conv2 3x3 + BN,
1x1/s2 shortcut conv + BN, add, sign) as a Bass/Tile kernel on 8 TRN2 cores.

Strategy:
- Data-parallel over batch: 16 images per core, weights/BN params replicated.
- Binarized weights are exactly +-1. x is split host-side into 2 fp16 terms
  (hi = fp16(x), lo = fp16(x - hi)) whose products with +-1 weights are
  exact, so conv1 accumulates x to ~2^-22 relative in fp32 PSUM. This is
  byte-optimal: the PE streams 256 B/cycle regardless of dtype, and fp16
  carries the most mantissa per byte (fp8 multi-term splits need >=5 terms
  for the required ~20 bits and lose). Shipping the split (same 4 B/elem
  as f32) instead of casting on-chip frees the scalar/vector engines.
- conv2 runs in fp8e4 DoubleRow mode (2x PE throughput): both operands are
  exactly +-1 (representable in e4m3), each pass contracts 2 cin tiles
  (256 channels), and accumulation is exact integers in fp32 PSUM.
- The shortcut uses the hi term only: its rounding feeds the final sign
  directly (no conv2 amplification), costing 275 deterministic flips
  (rel err 9.3e-3 < 2e-2), verified by exact CPU simulation.
- sign(clip(bn(z))) == sign(bn(z)): fused into one Sign activation with
  per-channel scale/bias APs. y is stored as fp8 (+-1 exact) and expanded
  to f32 on the host, quartering the output DMA.
- Consecutive matmuls never reuse a stationary weight tile (same-weight
  back-to-back serializes LDWEIGHTS; rotating weights pipelines it away).
- Conv taps stream fully-contiguous rhs and land at tap-dependent PSUM
  offsets (per-element has_written gives overwrite-then-accumulate):
  x is packed as stride-2 parity planes so every conv1 tap reads one
  contiguous span; conv2 streams whole unpadded a1 tiles into a 16x16
  PSUM window with garbage borders. Strided rhs streams ~20% slower
  (measured), and fatter SBUF-side traffic throttles the PE stream rate
  (~+22% per col at +45% traffic, measured), so rhs layouts stay lean
  and contiguous even at the cost of ~3% junk columns.
- Startup: w1 is laid out cout-tile-major so the first matmuls wait only
  on a 2.3KB/partition DMA chunk; group 0 orders taps by x-chunk
  availability (plane (0,0) first); the last group issues per-cout-tile
  y DMAs to shorten the drain tail.
"""

import numpy as np
import ml_dtypes
from contextlib import ExitStack

import concourse.tile as tile
from concourse import mybir, bacc
from concourse.bass_utils import run_bass_kernel_spmd

bf16 = ml_dtypes.bfloat16
f8e4 = ml_dtypes.float8_e4m3
F32 = mybir.dt.float32
BF = mybir.dt.bfloat16
F16 = mybir.dt.float16
F8 = mybir.dt.float8e4
DR = mybir.MatmulPerfMode.DoubleRow
SIGN = mybir.ActivationFunctionType.Sign
IDENT = mybir.ActivationFunctionType.Identity

N_CORES = 8
B, CIN, COUT, H = 128, 256, 512, 28
OH = 14                      # output spatial
BPC = B // N_CORES           # images per core
G = 2                        # images per matmul group
NG = BPC // G                # groups per core
NPG = G * OH * OH            # 392 valid pixels per group
NCT = COUT // 128            # cout tiles (4)
NCI1 = CIN // 128            # cin tiles for conv1/shortcut (2)
NCI2 = COUT // 128           # cin tiles for conv2 (4)
NSPL = 2                     # split terms for x (fp16 hi/lo)
EPS = np.float32(1e-5)

# parity-plane packing of the 29x29 zero-padded input (pad at index 0):
# plane (ph, pw) = xpad[2i+ph, 2j+pw]; heights/widths 15 or 14.
PL_W = {0: 15, 1: 14}  # plane widths by w-parity (heights analogous)
PL_OFF = {(0, 0): 0, (0, 1): 225, (1, 0): 435, (1, 1): 645}
XL = 841
XA = 225  # A-chunk boundary: plane (0,0) (taps 0,2,6,8)

_prog_cache = {}


def _build_program():
    nc = bacc.Bacc("TRN2", debug=False)

    xp = [nc.dram_tensor(f"xp{ci}", [128, BPC, NSPL, XL], F16,
                         kind="ExternalInput").ap() for ci in range(NCI1)]
    w1 = nc.dram_tensor("w1t", [128, NCT, NCI1, 9, 128], F16,
                    kind="ExternalInput").ap()
    w2 = nc.dram_tensor("w2t", [128, 9, NCI2, COUT], F8, kind="ExternalInput").ap()
    wsc = nc.dram_tensor("wsct", [128, NCI1, COUT], F16, kind="ExternalInput").ap()
    bnc = nc.dram_tensor("bnc", [128, 5, NCT], F32, kind="ExternalInput").ap()
    y = nc.dram_tensor("y", [128, NCT, BPC, OH * OH], F8,
                       kind="ExternalOutput").ap()

    with tile.TileContext(nc) as tc, ExitStack() as ctx:
        # few pools (tags share them): each pool exit costs an epilogue
        # barrier round (~1.2us), so 3 pools instead of 8
        consts = ctx.enter_context(tc.tile_pool(name="consts", bufs=1))
        work = ctx.enter_context(tc.tile_pool(name="work", bufs=1))
        pP = ctx.enter_context(tc.tile_pool(name="pP", bufs=4, space="PSUM"))

        w1_sb = consts.tile([128, NCT, NCI1, 9, 128], F16)
        wsc_sb = consts.tile([128, NCI1, COUT], F16)
        bnc_sb = consts.tile([128, 5, NCT], F32)
        w2_sb = consts.tile([128, 9, NCI2, COUT], F8)

        def bn_ap(i, c):
            return bnc_sb[:, i, c:c + 1]

        def split_tiles(gi):
            # hi/lo in separate [G, XL] tiles: a wider image-dim stride in
            # the rhs AP slows the PE stream ~25% (measured), so keep the
            # exact stride the fast layout had
            his = [work.tile([128, G, XL], F16, tag="spl", bufs=12,
                             name=f"hi_{gi}_{ci}") for ci in range(NCI1)]
            los = [work.tile([128, G, XL], F16, tag="spl", bufs=12,
                             name=f"lo_{gi}_{ci}") for ci in range(NCI1)]
            return his, los

        # group-0 startup: taps of parity plane (0,0) (x cols 0:225) can run
        # on chunk A alone, so interleave both ci's x chunks with only the
        # first two w1 chunks in the DMA queue, then split hi/lo per chunk
        A_TAPS = (0, 2, 6, 8)   # read plane (0,0) = cols 0:225
        B_TAPS = (1, 7, 3, 5, 4)  # read cols XA:XL

        sched = [(b0, G) for b0 in range(0, BPC, G)]
        for gi, (b0, gs) in enumerate(sched):
            bsl = slice(b0, b0 + gs)
            npg = gs * OH * OH
            # ---- load pre-split fp16 hi/lo terms (computed on host) ----
            his, los = split_tiles(gi)
            if gi == 0:
                # w1(0,0) first: LDWEIGHTS for the first matmul precedes the
                # rhs read, and the x chunk transfer overlaps it
                nc.sync.dma_start(w1_sb[:, 0, 0], w1[:, 0, 0])
                nc.sync.dma_start(his[0][:, 0:gs, 0:XA], xp[0][:, bsl, 0, 0:XA])
                nc.sync.dma_start(his[0][:, 0:gs, XA:XL],
                                  xp[0][:, bsl, 0, XA:XL])
                nc.sync.dma_start(los[0][:, 0:gs], xp[0][:, bsl, 1])
                nc.sync.dma_start(his[1][:, 0:gs, 0:XA], xp[1][:, bsl, 0, 0:XA])
                nc.sync.dma_start(w1_sb[:, 0, 1], w1[:, 0, 1])
                nc.sync.dma_start(his[1][:, 0:gs, XA:XL],
                                  xp[1][:, bsl, 0, XA:XL])
                nc.sync.dma_start(los[1][:, 0:gs], xp[1][:, bsl, 1])
                nc.sync.dma_start(wsc_sb[:], wsc[:])
                nc.sync.dma_start(bnc_sb[:], bnc[:])
                for c in range(1, NCT):
                    for cc in range(NCI1):
                        nc.sync.dma_start(w1_sb[:, c, cc], w1[:, c, cc])
                for blk in range(9):
                    nc.sync.dma_start(w2_sb[:, blk], w2[:, blk])
            else:
                for ci in range(NCI1):
                    nc.sync.dma_start(his[ci][:, 0:gs], xp[ci][:, bsl, 0])
                    nc.sync.dma_start(los[ci][:, 0:gs], xp[ci][:, bsl, 1])
            parts = list(zip(his, los))

            # ---- conv1 + interleaved shortcut matmuls ----
            # conv1 psum [128, gs, 14, 16]; valid cols 1..14
            p1, psc = [], []
            for c in range(NCT):
                pt = pP.tile([128, gs, 14, 16], F32, tag="pA", name=f"p1_{gi}_{c}")
                if gi == 0 and c == 0:
                    # chunk-availability order: A-plane taps of both split
                    # terms first, then B taps, per ci
                    order = [(s, ci, t) for ci in range(NCI1)
                             for taps in (A_TAPS, B_TAPS)
                             for s in range(NSPL) for t in taps]
                else:
                    order = [(s, ci, t) for s in range(NSPL)
                             for ci in range(NCI1) for t in range(9)]
                last = len(order) - 1
                for idx, (s, ci, t) in enumerate(order):
                    kh, kw = divmod(t, 3)
                    ph, pw = kh & 1, kw & 1
                    dh = 1 if kh == 2 else 0
                    # kh==0 taps read plane row 0 = the zero pad row;
                    # skip it (contributes exact zeros) -> 13 rows
                    r0 = 1 if kh == 0 else 0
                    nrows = 14 - r0
                    w_pl = PL_W[pw]
                    off = PL_OFF[(ph, pw)] + (dh + r0) * w_pl
                    c0 = 0 if kw == 2 else 1
                    w_ap = w1_sb[:, c, ci, t, :]
                    rhs = parts[ci][s][:, 0:gs, off:off + nrows * w_pl]
                    nc.tensor.matmul(
                        pt[:, :, r0:14, c0:c0 + w_pl], w_ap, rhs,
                        start=(idx == 0), stop=(idx == last))
                p1.append(pt)
                # shortcut for this cout tile: odd/odd parity plane.
                # hi term only: sc rounding feeds the final sign directly
                # (no conv2 amplification); costs ~275 deterministic flips
                # (rel err ~9e-3), verified by exact CPU simulation.
                st = pP.tile([128, NPG], F32, tag="pS", name=f"psc_{gi}_{c}")
                for ci in range(NCI1):
                    w_ap = wsc_sb[:, ci, c * 128:(c + 1) * 128]
                    rhs = parts[ci][0][:, 0:gs, 645:841]
                    nc.tensor.matmul(st[:, 0:npg], w_ap, rhs,
                                     start=(ci == 0), stop=(ci == NCI1 - 1))
                psc.append(st)

            # ---- a1 = sign(bn1(conv1)), unpadded fp8 ci-pair tiles ----
            a1 = []
            for j in range(NCI2 // 2):
                a1.append(work.tile([128, 2, G, OH, OH], F8, tag="a1", bufs=12,
                                    name=f"a1_{gi}_{j}"))
            for c in range(NCT):
                nc.scalar.activation(a1[c // 2][:, c % 2, 0:gs],
                                     p1[c][:, :, :, 1:15],
                                     SIGN, bias=bn_ap(1, c), scale=bn_ap(0, c))

            # ---- conv2: whole-a1 streams into shifted 16x16 psum window ----
            # psum [128, gs, 16, 16]; valid [1:15, 1:15]
            p2 = []
            for c in range(NCT):
                pt = pP.tile([128, gs, 16, 16], F32, tag="pA", name=f"p2_{gi}_{c}")
                idx, last = 0, NCI2 // 2 * 9 - 1
                for j in range(NCI2 // 2):
                    for t in range(9):
                        kh, kw = divmod(t, 3)
                        w_ap = w2_sb[:, t, 2 * j:2 * j + 2, c * 128:(c + 1) * 128]
                        out = pt[:, :, 2 - kh:16 - kh, 2 - kw:16 - kw]
                        nc.tensor.matmul(out, w_ap, a1[j][:, 0:2, 0:gs],
                                         start=(idx == 0), stop=(idx == last),
                                         perf_mode=DR)
                        idx += 1
                p2.append(pt)

            # ---- y = sign(scale2*p2 + shift2 + scalesc*psc + shiftsc) ----
            yt = work.tile([128, NCT, NPG], F8, tag="y", bufs=3,
                           name=f"y_{gi}")
            for c in range(NCT):
                # wt = scale2*p2 + (shift2+shiftsc): ACT handles the 4D
                # strided psum window; stt only takes 2D/3D operands.
                wt = work.tile([128, NPG], F32, tag="u", bufs=6,
                               name=f"u_{gi}_{c}")
                nc.scalar.activation(
                    wt[:, 0:npg].rearrange("p (b h w) -> p b h w",
                                           b=gs, h=OH, w=OH),
                    p2[c][:, :, 1:15, 1:15], IDENT,
                    bias=bn_ap(3, c), scale=bn_ap(2, c))
                vt = work.tile([128, NPG], F32, tag="v", bufs=3,
                               name=f"v_{gi}_{c}")
                nc.vector.scalar_tensor_tensor(
                    vt[:, 0:npg], psc[c][:, 0:npg], bn_ap(4, c), wt[:, 0:npg],
                    op0=mybir.AluOpType.mult, op1=mybir.AluOpType.add)
                nc.scalar.activation(yt[:, c, 0:npg], vt[:, 0:npg], SIGN)
                if gi == len(sched) - 1:
                    # tail: don't wait for all 4 cout tiles' sign acts
                    nc.sync.dma_start(
                        y[:, c, bsl].rearrange("p b x -> p (b x)"),
                        yt[:, c, 0:npg])
            if gi != len(sched) - 1:
                nc.sync.dma_start(
                    y[:, :, bsl].rearrange("p c b x -> p c (b x)"),
                    yt[:, :, 0:npg])

    nc.compile()
    return nc


def _prep_consts(w1, w2, wsc, g1, b1, m1, v1, g2, b2, m2, v2, gsc, bsc, msc, vsc):
    def sgn_w(w, dt):
        return np.where(w >= 0, np.float32(1.0), np.float32(-1.0)).astype(dt)

    # lhsT layouts: [cin_part(128), tap*NCI+ci, cout]
    w1s = sgn_w(w1, np.float16)  # [COUT, CIN, 3, 3]
    a1w = np.empty((128, NCT, NCI1, 9, 128), np.float16)
    for t in range(9):
        kh, kw = divmod(t, 3)
        for ci in range(NCI1):
            for c in range(NCT):
                a1w[:, c, ci, t, :] = w1s[c * 128:(c + 1) * 128,
                                          ci * 128:(ci + 1) * 128, kh, kw].T
    w2s = sgn_w(w2, f8e4)
    a2w = np.empty((128, 9, NCI2, COUT), f8e4)
    for t in range(9):
        kh, kw = divmod(t, 3)
        for ci in range(NCI2):
            a2w[:, t, ci, :] = w2s[:, ci * 128:(ci + 1) * 128, kh, kw].T
    wscs = sgn_w(wsc, np.float16)
    asw = np.empty((128, NCI1, COUT), np.float16)
    for ci in range(NCI1):
        asw[:, ci, :] = wscs[:, ci * 128:(ci + 1) * 128, 0, 0].T

    def bn_affine(g, b, m, v):
        scale = (g / np.sqrt(v + EPS)).astype(np.float32)
        shift = (b - m * g / np.sqrt(v + EPS)).astype(np.float32)
        return scale, shift

    sc1, sh1 = bn_affine(g1, b1, m1, v1)
    sc2, sh2 = bn_affine(g2, b2, m2, v2)
    scs, shs = bn_affine(gsc, bsc, msc, vsc)
    bnc = np.empty((128, 5, NCT), np.float32)
    for c in range(NCT):
        cs = slice(c * 128, (c + 1) * 128)
        bnc[:, 0, c] = sc1[cs]
        bnc[:, 1, c] = sh1[cs]
        bnc[:, 2, c] = sc2[cs]
        bnc[:, 3, c] = (sh2 + shs)[cs]
        bnc[:, 4, c] = scs[cs]
    return a1w, a2w, asw, bnc


def kernel(x, w1, g1, b1, m1, v1, w2, g2, b2, m2, v2, wsc, gsc, bsc, msc, vsc,
           _trace=False):
    x = np.ascontiguousarray(x, np.float32)
    a1w, a2w, asw, bnc = _prep_consts(
        np.asarray(w1, np.float32), np.asarray(w2, np.float32),
        np.asarray(wsc, np.float32),
        *[np.asarray(t, np.float32) for t in (g1, b1, m1, v1)],
        *[np.asarray(t, np.float32) for t in (g2, b2, m2, v2)],
        *[np.asarray(t, np.float32) for t in (gsc, bsc, msc, vsc)])

    # padded, channel-major x repacked as concatenated stride-2 parity planes
    xpad = np.zeros((CIN, B, H + 1, H + 1), np.float32)
    xpad[:, :, 1:, 1:] = x.transpose(1, 0, 2, 3)
    xflat = np.concatenate(
        [xpad[:, :, ph::2, pw::2].reshape(CIN, B, -1)
         for ph in (0, 1) for pw in (0, 1)], axis=2)  # [CIN, B, 841]

    # exact 2-term fp16 split, computed host-side (same RN casts the DVE
    # would do): ships the same 4 B/element as f32 but lands ready to stream
    xhi = xflat.astype(np.float16)
    xsplit = np.stack([xhi, (xflat - xhi.astype(np.float32))
                       .astype(np.float16)], axis=2)  # [CIN, B, 2, 841]

    if "nc" not in _prog_cache:
        _prog_cache["nc"] = _build_program()
    nc = _prog_cache["nc"]

    in_maps = []
    for k in range(N_CORES):
        m = {"w1t": a1w, "w2t": a2w, "wsct": asw, "bnc": bnc}
        for ci in range(NCI1):
            m[f"xp{ci}"] = np.ascontiguousarray(
                xsplit[ci * 128:(ci + 1) * 128, k * BPC:(k + 1) * BPC])
        in_maps.append(m)

    res = run_bass_kernel_spmd(nc, in_maps, core_ids=list(range(N_CORES)),
                               trace=_trace)

    # y dram: [128, NCT, BPC, 196] per core -> [B, COUT, 14, 14]
    out = np.empty((B, COUT, OH, OH), np.float32)
    for k in range(N_CORES):
        yk = res.results[k]["y"].astype(np.float32)  # fp8 +-1 -> f32
        out[k * BPC:(k + 1) * BPC] = (
            yk.transpose(2, 1, 0, 3).reshape(BPC, COUT, OH, OH))
    if _trace:
        kernel.last_results = res
    return out



# revision 20
# speedup vs baseline: 1.1917x; 1.1917x over previous
"""Binarized ResNet BasicBlock (conv1 3x3/s2 + BN + sign, conv2 3x3 + BN,
1x1/s2 shortcut conv + BN, add, sign) as a Bass/Tile kernel on 8 TRN2 cores.

Strategy:
- Data-parallel over batch: 16 images per core, weights/BN params replicated.
- Binarized weights are exactly +-1. x is split host-side into 2 fp16 terms
  (hi = fp16(x), lo = fp16(x - hi)) whose products with +-1 weights are
  exact, so conv1 accumulates x to ~2^-22 relative in fp32 PSUM. This is
  byte-optimal: the PE streams 256 B/cycle regardless of dtype, and fp16
  carries the most mantissa per byte (fp8 multi-term splits need >=5 terms
  for the required ~20 bits and lose). Shipping the split (same 4 B/elem
  as f32) instead of casting on-chip frees the scalar/vector engines.
- conv2 runs in fp8e4 DoubleRow mode (2x PE throughput): both operands are
  exactly +-1 (representable in e4m3), each pass contracts 2 cin tiles
  (256 channels), and accumulation is exact integers in fp32 PSUM.
- The shortcut uses the hi term only: its rounding feeds the final sign
  directly (no conv2 amplification), costing 275 deterministic flips
  (rel err 9.3e-3 < 2e-2), verified by exact CPU simulation.
- sign(clip(bn(z))) == sign(bn(z)): fused into one Sign activation with
  per-channel scale/bias APs. y is stored as fp8 (+-1 exact) and expanded
  to f32 on the host, quartering the output DMA.
- Consecutive matmuls never reuse a stationary weight tile (same-weight
  back-to-back serializes LDWEIGHTS; rotating weights pipelines it away).
- Conv taps stream fully-contiguous rhs and land at tap-dependent PSUM
  offsets (per-element has_written gives overwrite-then-accumulate):
  x is packed as stride-2 parity planes so every conv1 tap reads one
  contiguous span; conv2 streams whole unpadded a1 tiles into a 16x16
  PSUM window with garbage borders. Strided rhs streams ~20% slower
  (measured), and fatter SBUF-side traffic throttles the PE stream rate
  (~+22% per col at +45% traffic, measured), so rhs layouts stay lean
  and contiguous even at the cost of ~3% junk columns.
- Startup: w1 is laid out cout-tile-major so the first matmuls wait only
  on a 2.3KB/partition DMA chunk; group 0 orders taps by x-chunk
  availability (plane (0,0) first); the last group issues per-cout-tile
  y DMAs to shorten the drain tail.
"""

import numpy as np
import ml_dtypes
from contextlib import ExitStack

import concourse.tile as tile
from concourse import mybir, bacc
from concourse.bass_utils import run_bass_kernel_spmd

bf16 = ml_dtypes.bfloat16
f8e4 = ml_dtypes.float8_e4m3
F32 = mybir.dt.float32
BF = mybir.dt.bfloat16
F16 = mybir.dt.float16
F8 = mybir.dt.float8e4
DR = mybir.MatmulPerfMode.DoubleRow
SIGN = mybir.ActivationFunctionType.Sign
IDENT = mybir.ActivationFunctionType.Identity

N_CORES = 8
B, CIN, COUT, H = 128, 256, 512, 28
OH = 14                      # output spatial
BPC = B // N_CORES           # images per core
G = 2                        # images per matmul group
NG = BPC // G                # groups per core
NPG = G * OH * OH            # 392 valid pixels per group
NCT = COUT // 128            # cout tiles (4)
NCI1 = CIN // 128            # cin tiles for conv1/shortcut (2)
NCI2 = COUT // 128           # cin tiles for conv2 (4)
NSPL = 2                     # split terms for x (fp16 hi/lo)
EPS = np.float32(1e-5)

# parity-plane packing of the 29x29 zero-padded input (pad at index 0):
# plane (ph, pw) = xpad[2i+ph, 2j+pw]; heights/widths 15 or 14.
PL_W = {0: 15, 1: 14}  # plane widths by w-parity (heights analogous)
PL_OFF = {(0, 0): 0, (0, 1): 225, (1, 0): 435, (1, 1): 645}
XL = 841
XA = 225  # A-chunk boundary: plane (0,0) (taps 0,2,6,8)

_prog_cache = {}


def _build_program():
    nc = bacc.Bacc("TRN2", debug=False)

    xp = [nc.dram_tensor(f"xp{ci}", [128, BPC, NSPL, XL], F16,
                         kind="ExternalInput").ap() for ci in range(NCI1)]
    w1 = nc.dram_tensor("w1t", [128, NCT, NCI1, 9, 128], F16,
                    kind="ExternalInput").ap()
    w2 = nc.dram_tensor("w2t", [128, 9, NCI2, COUT], F8, kind="ExternalInput").ap()
    wsc = nc.dram_tensor("wsct", [128, NCI1, COUT], F16, kind="ExternalInput").ap()
    bnc = nc.dram_tensor("bnc", [128, 5, NCT], F32, kind="ExternalInput").ap()
    y = nc.dram_tensor("y", [128, NCT, BPC, OH * OH], F8,
                       kind="ExternalOutput").ap()

    with tile.TileContext(nc) as tc, ExitStack() as ctx:
        # few pools (tags share them): each pool exit costs an epilogue
        # barrier round (~1.2us), so 3 pools instead of 8
        consts = ctx.enter_context(tc.tile_pool(name="consts", bufs=1))
        work = ctx.enter_context(tc.tile_pool(name="work", bufs=1))
        pP = ctx.enter_context(tc.tile_pool(name="pP", bufs=4, space="PSUM"))

        w1_sb = consts.tile([128, NCT, NCI1, 9, 128], F16)
        wsc_sb = consts.tile([128, NCI1, COUT], F16)
        bnc_sb = consts.tile([128, 5, NCT], F32)
        w2_sb = consts.tile([128, 9, NCI2, COUT], F8)

        def bn_ap(i, c):
            return bnc_sb[:, i, c:c + 1]

        def split_tiles(gi):
            # hi/lo in separate [G, XL] tiles: a wider image-dim stride in
            # the rhs AP slows the PE stream ~25% (measured), so keep the
            # exact stride the fast layout had
            his = [work.tile([128, G, XL], F16, tag="spl", bufs=12,
                             name=f"hi_{gi}_{ci}") for ci in range(NCI1)]
            los = [work.tile([128, G, XL], F16, tag="spl", bufs=12,
                             name=f"lo_{gi}_{ci}") for ci in range(NCI1)]
            return his, los

        # group-0 startup: taps of parity plane (0,0) (x cols 0:225) can run
        # on chunk A alone, so interleave both ci's x chunks with only the
        # first two w1 chunks in the DMA queue, then split hi/lo per chunk
        A_TAPS = (0, 2, 6, 8)   # read plane (0,0) = cols 0:225
        B_TAPS = (1, 7, 3, 5, 4)  # read cols XA:XL

        sched = [(b0, G) for b0 in range(0, BPC, G)]
        for gi, (b0, gs) in enumerate(sched):
            bsl = slice(b0, b0 + gs)
            npg = gs * OH * OH
            # ---- load pre-split fp16 hi/lo terms (computed on host) ----
            his, los = split_tiles(gi)
            if gi == 0:
                nc.sync.dma_start(his[0][:, 0:gs, 0:XA], xp[0][:, bsl, 0, 0:XA])
                nc.sync.dma_start(w1_sb[:, 0, 0], w1[:, 0, 0])
                nc.sync.dma_start(his[0][:, 0:gs, XA:XL],
                                  xp[0][:, bsl, 0, XA:XL])
                nc.sync.dma_start(los[0][:, 0:gs], xp[0][:, bsl, 1])
                nc.sync.dma_start(his[1][:, 0:gs, 0:XA], xp[1][:, bsl, 0, 0:XA])
                nc.sync.dma_start(w1_sb[:, 0, 1], w1[:, 0, 1])
                nc.sync.dma_start(his[1][:, 0:gs, XA:XL],
                                  xp[1][:, bsl, 0, XA:XL])
                nc.sync.dma_start(los[1][:, 0:gs], xp[1][:, bsl, 1])
                nc.sync.dma_start(wsc_sb[:], wsc[:])
                nc.sync.dma_start(bnc_sb[:], bnc[:])
                for c in range(1, NCT):
                    for cc in range(NCI1):
                        nc.sync.dma_start(w1_sb[:, c, cc], w1[:, c, cc])
                for blk in range(9):
                    nc.sync.dma_start(w2_sb[:, blk], w2[:, blk])
            else:
                for ci in range(NCI1):
                    nc.sync.dma_start(his[ci][:, 0:gs], xp[ci][:, bsl, 0])
                    nc.sync.dma_start(los[ci][:, 0:gs], xp[ci][:, bsl, 1])
            parts = list(zip(his, los))

            # ---- conv1 + interleaved shortcut matmuls ----
            # conv1 psum [128, gs, 14, 16]; valid cols 1..14
            p1, psc = [], []
            for c in range(NCT):
                pt = pP.tile([128, gs, 14, 16], F32, tag="pA", name=f"p1_{gi}_{c}")
                if gi == 0 and c == 0:
                    # chunk-availability order: A-plane taps of both split
                    # terms first, then B taps, per ci
                    order = [(s, ci, t) for ci in range(NCI1)
                             for taps in (A_TAPS, B_TAPS)
                             for s in range(NSPL) for t in taps]
                else:
                    order = [(s, ci, t) for s in range(NSPL)
                             for ci in range(NCI1) for t in range(9)]
                last = len(order) - 1
                for idx, (s, ci, t) in enumerate(order):
                    kh, kw = divmod(t, 3)
                    ph, pw = kh & 1, kw & 1
                    dh = 1 if kh == 2 else 0
                    # kh==0 taps read plane row 0 = the zero pad row;
                    # skip it (contributes exact zeros) -> 13 rows
                    r0 = 1 if kh == 0 else 0
                    nrows = 14 - r0
                    w_pl = PL_W[pw]
                    off = PL_OFF[(ph, pw)] + (dh + r0) * w_pl
                    c0 = 0 if kw == 2 else 1
                    w_ap = w1_sb[:, c, ci, t, :]
                    rhs = parts[ci][s][:, 0:gs, off:off + nrows * w_pl]
                    nc.tensor.matmul(
                        pt[:, :, r0:14, c0:c0 + w_pl], w_ap, rhs,
                        start=(idx == 0), stop=(idx == last))
                p1.append(pt)
                # shortcut for this cout tile: odd/odd parity plane.
                # hi term only: sc rounding feeds the final sign directly
                # (no conv2 amplification); costs ~275 deterministic flips
                # (rel err ~9e-3), verified by exact CPU simulation.
                st = pP.tile([128, NPG], F32, tag="pS", name=f"psc_{gi}_{c}")
                for ci in range(NCI1):
                    w_ap = wsc_sb[:, ci, c * 128:(c + 1) * 128]
                    rhs = parts[ci][0][:, 0:gs, 645:841]
                    nc.tensor.matmul(st[:, 0:npg], w_ap, rhs,
                                     start=(ci == 0), stop=(ci == NCI1 - 1))
                psc.append(st)

            # ---- a1 = sign(bn1(conv1)), unpadded fp8 ci-pair tiles ----
            a1 = []
            for j in range(NCI2 // 2):
                a1.append(work.tile([128, 2, G, OH, OH], F8, tag="a1", bufs=12,
                                    name=f"a1_{gi}_{j}"))
            for c in range(NCT):
                nc.scalar.activation(a1[c // 2][:, c % 2, 0:gs],
                                     p1[c][:, :, :, 1:15],
                                     SIGN, bias=bn_ap(1, c), scale=bn_ap(0, c))

            # ---- conv2: whole-a1 streams into shifted 16x16 psum window ----
            # psum [128, gs, 16, 16]; valid [1:15, 1:15]
            p2 = []
            for c in range(NCT):
                pt = pP.tile([128, gs, 16, 16], F32, tag="pA", name=f"p2_{gi}_{c}")
                idx, last = 0, NCI2 // 2 * 9 - 1
                for j in range(NCI2 // 2):
                    for t in range(9):
                        kh, kw = divmod(t, 3)
                        w_ap = w2_sb[:, t, 2 * j:2 * j + 2, c * 128:(c + 1) * 128]
                        out = pt[:, :, 2 - kh:16 - kh, 2 - kw:16 - kw]
                        nc.tensor.matmul(out, w_ap, a1[j][:, 0:2, 0:gs],
                                         start=(idx == 0), stop=(idx == last),
                                         perf_mode=DR)
                        idx += 1
                p2.append(pt)

            # ---- y = sign(scale2*p2 + shift2 + scalesc*psc + shiftsc) ----
            yt = work.tile([128, NCT, NPG], F8, tag="y", bufs=3,
                           name=f"y_{gi}")
            for c in range(NCT):
                # wt = scale2*p2 + (shift2+shiftsc): ACT handles the 4D
                # strided psum window; stt only takes 2D/3D operands.
                wt = work.tile([128, NPG], F32, tag="u", bufs=6,
                               name=f"u_{gi}_{c}")
                nc.scalar.activation(
                    wt[:, 0:npg].rearrange("p (b h w) -> p b h w",
                                           b=gs, h=OH, w=OH),
                    p2[c][:, :, 1:15, 1:15], IDENT,
                    bias=bn_ap(3, c), scale=bn_ap(2, c))
                vt = work.tile([128, NPG], F32, tag="v", bufs=3,
                               name=f"v_{gi}_{c}")
                nc.vector.scalar_tensor_tensor(
                    vt[:, 0:npg], psc[c][:, 0:npg], bn_ap(4, c), wt[:, 0:npg],
                    op0=mybir.AluOpType.mult, op1=mybir.AluOpType.add)
                nc.scalar.activation(yt[:, c, 0:npg], vt[:, 0:npg], SIGN)
                if gi == len(sched) - 1:
                    # tail: don't wait for all 4 cout tiles' sign acts
                    nc.sync.dma_start(
                        y[:, c, bsl].rearrange("p b x -> p (b x)"),
                        yt[:, c, 0:npg])
            if gi != len(sched) - 1:
                nc.sync.dma_start(
                    y[:, :, bsl].rearrange("p c b x -> p c (b x)"),
                    yt[:, :, 0:npg])

    nc.compile()
    return nc


def _prep_consts(w1, w2, wsc, g1, b1, m1, v1, g2, b2, m2, v2, gsc, bsc, msc, vsc):
    def sgn_w(w, dt):
        return np.where(w >= 0, np.float32(1.0), np.float32(-1.0)).astype(dt)

    # lhsT layouts: [cin_part(128), tap*NCI+ci, cout]
    w1s = sgn_w(w1, np.float16)  # [COUT, CIN, 3, 3]
    a1w = np.empty((128, NCT, NCI1, 9, 128), np.float16)
    for t in range(9):
        kh, kw = divmod(t, 3)
        for ci in range(NCI1):
            for c in range(NCT):
                a1w[:, c, ci, t, :] = w1s[c * 128:(c + 1) * 128,
                                          ci * 128:(ci + 1) * 128, kh, kw].T
    w2s = sgn_w(w2, f8e4)
    a2w = np.empty((128, 9, NCI2, COUT), f8e4)
    for t in range(9):
        kh, kw = divmod(t, 3)
        for ci in range(NCI2):
            a2w[:, t, ci, :] = w2s[:, ci * 128:(ci + 1) * 128, kh, kw].T
    wscs = sgn_w(wsc, np.float16)
    asw = np.empty((128, NCI1, COUT), np.float16)
    for ci in range(NCI1):
        asw[:, ci, :] = wscs[:, ci * 128:(ci + 1) * 128, 0, 0].T

    def bn_affine(g, b, m, v):
        scale = (g / np.sqrt(v + EPS)).astype(np.float32)
        shift = (b - m * g / np.sqrt(v + EPS)).astype(np.float32)
        return scale, shift

    sc1, sh1 = bn_affine(g1, b1, m1, v1)
    sc2, sh2 = bn_affine(g2, b2, m2, v2)
    scs, shs = bn_affine(gsc, bsc, msc, vsc)
    bnc = np.empty((128, 5, NCT), np.float32)
    for c in range(NCT):
        cs = slice(c * 128, (c + 1) * 128)
        bnc[:, 0, c] = sc1[cs]
        bnc[:, 1, c] = sh1[cs]
        bnc[:, 2, c] = sc2[cs]
        bnc[:, 3, c] = (sh2 + shs)[cs]
        bnc[:, 4, c] = scs[cs]
    return a1w, a2w, asw, bnc


def kernel(x, w1, g1, b1, m1, v1, w2, g2, b2, m2, v2, wsc, gsc, bsc, msc, vsc,
           _trace=False):
    x = np.ascontiguousarray(x, np.float32)
    a1w, a2w, asw, bnc = _prep_consts(
        np.asarray(w1, np.float32), np.asarray(w2, np.float32),
        np.asarray(wsc, np.float32),
        *[np.asarray(t, np.float32) for t in (g1, b1, m1, v1)],
        *[np.asarray(t, np.float32) for t in (g2, b2, m2, v2)],
        *[np.asarray(t, np.float32) for t in (gsc, bsc, msc, vsc)])

    # padded, channel-major x repacked as concatenated stride-2 parity planes
    xpad = np.zeros((CIN, B, H + 1, H + 1), np.float32)
    xpad[:, :, 1:, 1:] = x.transpose(1, 0, 2, 3)
    xflat = np.concatenate(
        [xpad[:, :, ph::2, pw::2].reshape(CIN, B, -1)
         for ph in (0, 1) for pw in (0, 1)], axis=2)  # [CIN, B, 841]

    # exact 2-term fp16 split, computed host-side (same RN casts the DVE
    # would do): ships the same 4 B/element as f32 but lands ready to stream
    xhi = xflat.astype(np.float16)
    xsplit = np.stack([xhi, (xflat - xhi.astype(np.float32))
                       .astype(np.float16)], axis=2)  # [CIN, B, 2, 841]

    if "nc" not in _prog_cache:
        _prog_cache["nc"] = _build_program()
    nc = _prog_cache["nc"]

    in_maps = []
    for k in range(N_CORES):
        m = {"w1t": a1w, "w2t": a2w, "wsct": asw, "bnc": bnc}
        for ci in range(NCI1):
            m[f"xp{ci}"] = np.ascontiguousarray(
                xsplit[ci * 128:(ci + 1) * 128, k * BPC:(k + 1) * BPC])
        in_maps.append(m)

    res = run_bass_kernel_spmd(nc, in_maps, core_ids=list(range(N_CORES)),
                               trace=_trace)

    # y dram: [128, NCT, BPC, 196] per core -> [B, COUT, 14, 14]
    out = np.empty((B, COUT, OH, OH), np.float32)
    for k in range(N_CORES):
        yk = res.results[k]["y"].astype(np.float32)  # fp8 +-1 -> f32
        out[k * BPC:(k + 1) * BPC] = (
            yk.transpose(2, 1, 0, 3).reshape(BPC, COUT, OH, OH))
    if _trace:
        kernel.last_results = res
    return out

